# revision 45
# baseline (speedup 1.0000x reference)
"""Trainium2 Bass kernel for nn_BPBookLayer (retrieval_knn).

Computation (per full input):
  query = mean(x, axis=1)                         [B, D]
  scores = cos_sim(query, prototypes)             [B, P]
  top5 -> softmax -> agg = attn @ protos[top5]    [B, D]
  out = x + 0.1 * agg[:, None, :]

Sharding: data-parallel over batch B=32 across 8 cores (4 batches/core),
prototypes replicated.

All DMA serializes on one ~360 GB/s device charged by out-side bytes, so
the kernel is bound by x in fp16 (16.8 MB) + out fp16 (16.8 MB) + the
1 MB fp8 proto table: ~96.4 us of transfer. Everything else is scheduled
to hide under that stream; the layout below keeps the store producer
ahead of the drain with zero mid-stream starvation beyond ~2 us.

Per-core design:
 - x loads f32 HBM -> fp16 SBUF (gpsimd casting DMA) as 8 whole-half-
   batch tiles [128, 8*D] (TROWS=1024): only 10 SWDGE DMAs total, so
   every load's desc-gen completes (and claims its DMA-engine FIFO slot)
   within ~12 us -- stores can never jump ahead of pending loads and
   push the last batch's tiles (and its whole chain) later.
 - prototypes load fp8-e4m3 in two early bulk DMAs (bulk first: its
   desc-gen starts at t=0); a 32-row f32 HWDGE head covers the window
   before SWDGE dispatches. 32 = SBUF quadrant size: engine ops on the
   [32:128] remainder must start on a 32-partition boundary (and span
   one quadrant) or the BIR verifier rejects them.
 - out stores fp16 (host upcasts); each batch is stored in 2-subrow
   quarters right behind its residual adds so production stays granular.
 - protoT = diag(1/||p||)-scaled transposes of the fp8 chunks (fp8 lhsT
   x fp16 diag through the f32 PE path), 4 chunks per PSUM bank. Norms:
   Act Square-accumulate straight off the fp8 table, then
   1/||p|| = exp(-0.5*ln(sumsq)) -- with Copy/Square/Ln/Exp all in ONE
   act-func set (explicit LoadActFuncSet: the auto-insert pass is greedy
   per-function and would thrash tables between Ln and Exp).
 - query accumulation: batches 0-2 on DVE (fp16 tensor_tensor folds of
   each tile into a [128, 4*D] accumulator, gated per tile), batch 3 on
   PE (ones-column matmuls riding the loads) so the post-load DVE window
   holds only adds + chain work. chain(2) is emitted before loads(3) so
   the chain cascade never queues behind batch-3-gated ops.
 - per-batch chain: scores row = sum_dc qT[:,dc].T @ protoT_n[dc]
   (16 matmuls); e = exp(cos/||q||) in fp16 and top-5/threshold/softmax
   all happen in e-space (exp is monotonic, halving DVE top-k cost);
   wt quantizes to fp8 while replicating across 128 columns (Act
   per-partition-scale copy), and agg+broadcast fuse into 8 fp8
   DoubleRow matmuls producing the [128, D] broadcast tile directly
   (M=128 makes DoubleRow legal; the 0.1/softmax-denominator scale is
   applied via a PE-broadcast coefficient column on the PSUM copy-out).
 - residual adds in-place on DVE fp16; each batch's adds are emitted
   after the NEXT chain (pending trick) so the DVE sequencer reaches
   chain ops first; the tail quarters of late batches ride the gpsimd
   engine in parallel.
"""

from contextlib import ExitStack

import numpy as np

import concourse.bacc as bacc
import concourse.bass as bass
import concourse.tile as tile
from concourse import mybir
from concourse.bass_utils import run_bass_kernel_spmd
from concourse.masks import make_identity

F32 = mybir.dt.float32
F16 = mybir.dt.float16
F8 = mybir.dt.float8e4
AF = mybir.ActivationFunctionType
ALU = mybir.AluOpType

B, L, D, P = 32, 2048, 1024, 1024
NCORES = 8
BLOC = B // NCORES  # batches per core
TROWS = 1024  # L rows per x tile
TSUB = TROWS // 128
NT = L // TROWS     # x tiles per batch
DCH = D // 128      # d chunks
PCH = P // 128      # p chunks
XBUFS = BLOC * NT   # all x tiles resident
ALPHA = 0.1
NHEAD = 32          # proto rows riding the f32 HWDGE head transfer
# (32 = SBUF partition quadrant: engine ops on the [NHEAD:128] remainder
#  must start on a 32-partition boundary or the BIR verifier rejects them)
POOL_ADDS = True
PROTOS_FIRST = True
FINE_ADDS = True
NO_PENDING = False
PE_Q3 = True      # batch-3 query reduction on PE instead of DVE
POOL_B3_EXTRA = False  # gpsimd also takes batch-3's second-to-last tile


def _ln_exp_set_id(arch):
    """Index of the act-func set holding Ln+Exp+Square+Copy together."""
    try:
        from concourse.hw_specs import get_activation_tables

        need = {AF.Ln, AF.Exp, AF.Square, AF.Copy}
        for i, funcs in enumerate(get_activation_tables(arch).values()):
            if need <= funcs:
                return i
    except Exception:
        pass
    return 6  # natural_log_exp_and_others in the stock act_info.json


def _kernel(tc, ctx, x, protos, out, repeat=1):
    nc = tc.nc

    singles = ctx.enter_context(tc.tile_pool(name="singles", bufs=1))
    xp = ctx.enter_context(tc.tile_pool(name="xp", bufs=XBUFS))
    sm = ctx.enter_context(tc.tile_pool(name="sm", bufs=2))
    ps4 = ctx.enter_context(tc.tile_pool(name="ps4", bufs=2, space="PSUM"))
    psq = ctx.enter_context(tc.tile_pool(name="psq", bufs=2, space="PSUM"))
    ps_bc = ctx.enter_context(tc.tile_pool(name="ps_bc", bufs=1, space="PSUM"))

    for _rep in range(repeat):
        proto8 = singles.tile([128, PCH, D], F8)
        protoT_sb = singles.tile([128, DCH, P], F16)  # normalized transpose
        inv_pnorm = singles.tile([128, PCH], F32)
        pnorm_sq = singles.tile([128, PCH], F32)

        ident = singles.tile([128, 128], F16)
        ones128 = singles.tile([128, 128], F16)
        e8 = singles.tile([1, 8, 8], F16)
        ones_col = singles.tile([128, 1], F16)
        ones_row = singles.tile([1, 128], F16)

        # one act table for the whole kernel (Copy/Square/Ln/Exp)
        nc.scalar.add_instruction(
            mybir.InstLoadActFuncSet(
                name=nc.get_next_instruction_name(),
                ins=[],
                outs=[],
                act_func_set_id=_ln_exp_set_id(nc.m.arch),
            )
        )

        # ---- f32 proto head on HWDGE (covers SWDGE desc-gen startup),
        # then the whole fp8 proto table in two early SWDGE transfers ----
        proto0_f32 = singles.tile([NHEAD, D], F32)
        nc.sync.dma_start(out=proto0_f32, in_=protos[0:NHEAD, :])
        # agg reads the fp8 table incl. the head rows
        nc.scalar.copy(out=proto8[0:NHEAD, 0, :], in_=proto0_f32)

        sq_scratch = sm.tile([128, D], F16, tag="sqs", bufs=1)

        def emit_sumsq(c):
            """squared norms on Act straight from the fp8 table"""
            if c == 0:
                nc.scalar.activation(
                    out=sq_scratch[0:NHEAD, :], in_=proto0_f32, func=AF.Square,
                    accum_out=pnorm_sq[0:NHEAD, 0:1],
                )
                for q0 in range(NHEAD, 128, 32):  # quadrant-sized slices:
                    # engine APs with base partition != 0 may span <= 32
                    nc.scalar.activation(
                        out=sq_scratch[q0 : q0 + 32, :],
                        in_=proto8[q0 : q0 + 32, 0, :],
                        func=AF.Square, accum_out=pnorm_sq[q0 : q0 + 32, 0:1],
                    )
            else:
                nc.scalar.activation(
                    out=sq_scratch, in_=proto8[:, c, :], func=AF.Square,
                    accum_out=pnorm_sq[:, c : c + 1],
                )

        def emit_inv_pnorm():
            """1/||p|| per partition: ln/exp on Act, all chunks at once."""
            nc.scalar.activation(out=pnorm_sq, in_=pnorm_sq, func=AF.Ln)
            nc.scalar.activation(
                out=inv_pnorm, in_=pnorm_sq, func=AF.Exp, scale=-0.5
            )

        xt = [[None] * NT for _ in range(BLOC)]
        acc = [None] * BLOC

        pe_q = {}

        def load_tile(b, i):
            t_ = xp.tile([128, TSUB * D], F16, tag="x", name=f"x{b}_{i}")
            xt[b][i] = t_
            nc.gpsimd.dma_start(
                out=t_,
                in_=x[b, TROWS * i : TROWS * (i + 1), :].rearrange(
                    "(p t) d -> p (t d)", p=128
                ),
            )
            if b == BLOC - 1 and PE_Q3:
                # last batch: query accumulates on PE (tile-gated matmuls)
                # so the post-load DVE window holds only adds + chain bits
                if i == 0:
                    pe_q[b] = [
                        psq.tile([1, 512], F32, tag="q", bufs=4,
                                 name=f"ps_q{b}_{h}")
                        for h in range(2)
                    ]
                for t in range(TSUB):
                    for h in range(2):
                        nc.tensor.matmul(
                            pe_q[b][h],
                            lhsT=ones_col,
                            rhs=xt[b][i][:, t * D + h * 512 : t * D + h * 512 + 512],
                            start=(i == 0 and t == 0),
                            stop=(i == NT - 1 and t == TSUB - 1),
                        )
                return
            # earlier batches: query accumulation rides DVE behind the
            # loads, folding each 8-subrow tile into a [128, 4*D] accumulator
            HALF = TSUB * D // 2
            if i == 0:
                a_ = sm.tile([128, HALF], F16, tag="qacc", bufs=2,
                             name=f"acc{b}")
                acc[b] = a_
                nc.vector.tensor_add(a_, t_[:, 0:HALF], t_[:, HALF:])
            else:
                nc.vector.tensor_add(acc[b], acc[b], t_[:, 0:HALF])
                nc.vector.tensor_add(acc[b], acc[b], t_[:, HALF:])

        def load_batch(b, skip_first=False):
            for i in range(1 if skip_first else 0, NT):
                load_tile(b, i)

        def emit_protoT(c):
            # diag(1/||p||)-scaled transposes of the fp8 chunk (fp8 lhsT x
            # fp16 diag is exact through the f32 PE path), 4 per PSUM bank
            diag_c = sm.tile([128, 128], F16, tag="diag", bufs=2, name=f"dg{c}")
            nc.scalar.activation(
                out=diag_c, in_=ident, func=AF.Copy,
                scale=inv_pnorm[:, c : c + 1],
            )
            for half in range(2):
                pst = ps4.tile([128, 4, 128], F32, tag="ps")
                for j in range(4):
                    dc = half * 4 + j
                    nc.tensor.matmul(
                        pst[:, j, :],
                        lhsT=proto8[:, c, dc * 128 : (dc + 1) * 128],
                        rhs=diag_c,
                        start=(j == 0),
                        stop=(j == 3),
                        skip_group_check=True,
                    )
                dst = protoT_sb[:, half * 4 : half * 4 + 4, c * 128 : (c + 1) * 128]
                if half == 0:
                    nc.scalar.copy(out=dst, in_=pst)
                else:
                    nc.vector.tensor_copy(dst, pst)

        q_sbs, qsqs = [], []

        def fold_q(b):
            if b in pe_q:
                ps_q = pe_q[b]  # accumulated during the loads
            else:
                ps_q = [
                    psq.tile([1, 512], F32, tag="q", bufs=4, name=f"ps_q{b}_{h}")
                    for h in range(2)
                ]
                for g in range(TSUB // 2):
                    for h in range(2):
                        nc.tensor.matmul(
                            ps_q[h],
                            lhsT=ones_col,
                            rhs=acc[b][:, g * D + h * 512 : g * D + h * 512 + 512],
                            start=(g == 0),
                            stop=(g == TSUB // 2 - 1),
                        )
            q_sb = sm.tile([1, D], F16, tag="q", bufs=2, name=f"q_sb{b}")
            for h in range(2):
                nc.scalar.copy(out=q_sb[0:1, h * 512 : (h + 1) * 512], in_=ps_q[h])
            qsq_sc = sm.tile([1, 512], F16, tag="qsq_sc", bufs=1)
            qsq = sm.tile([1, 2], F32, tag="qsq", bufs=2, name=f"qsq{b}")
            for h in range(2):
                nc.scalar.activation(
                    out=qsq_sc, in_=ps_q[h], func=AF.Square,
                    accum_out=qsq[0:1, h : h + 1],
                )
            q_sbs.append(q_sb)
            qsqs.append(qsq)

        def emit_adds(b, bc_h):
            bc1 = bc_h.rearrange("p (o d) -> p o d", o=1)
            for i in range(NT):
                xv = xt[b][i].rearrange("p (t d) -> p t d", d=D)
                hbm = out[b, TROWS * i : TROWS * (i + 1), :].rearrange(
                    "(p t) d -> p t d", p=128
                )
                if i == 0:
                    # first tile in 2-subrow quarters: stores launch early
                    for g in range(4):
                        sl = slice(2 * g, 2 * g + 2)
                        nc.vector.tensor_tensor(
                            out=xv[:, sl, :], in0=xv[:, sl, :],
                            in1=bc1.to_broadcast([128, 2, D]), op=ALU.add,
                        )
                        nc.sync.dma_start(
                            out=hbm[:, sl, :],
                            in_=xt[b][i][:, 2 * g * D : (2 * g + 2) * D],
                        )
                elif FINE_ADDS:
                    for g in range(4):
                        sl = slice(2 * g, 2 * g + 2)
                        # tail quarters ride gpsimd for late batches; Pool is
                        # in-order, so any pool op emitted before the last
                        # load desc-gen would stall it -- batch 1's pool
                        # quarters are deferred until after load_batch(3)
                        pool_this = POOL_ADDS and b >= 2 and g >= 2
                        if pool_this and b == 1:
                            deferred_pool.append(
                                (xv[:, sl, :], bc1,
                                 hbm[:, sl, :],
                                 xt[b][i][:, 2 * g * D : (2 * g + 2) * D])
                            )
                            continue
                        eng = nc.gpsimd if pool_this else nc.vector
                        eng.tensor_tensor(
                            out=xv[:, sl, :], in0=xv[:, sl, :],
                            in1=bc1.to_broadcast([128, 2, D]), op=ALU.add,
                        )
                        nc.sync.dma_start(
                            out=hbm[:, sl, :],
                            in_=xt[b][i][:, 2 * g * D : (2 * g + 2) * D],
                        )
                else:
                    for g in range(2):
                        sl = slice(4 * g, 4 * g + 4)
                        # second half rides gpsimd for late batches (their
                        # adds are emitted after every load desc-gen)
                        pool_this = POOL_ADDS and b >= 2 and g == 1
                        eng = nc.gpsimd if pool_this else nc.vector
                        eng.tensor_tensor(
                            out=xv[:, sl, :], in0=xv[:, sl, :],
                            in1=bc1.to_broadcast([128, 4, D]), op=ALU.add,
                        )
                        nc.sync.dma_start(
                            out=hbm[:, sl, :],
                            in_=xt[b][i][:, 4 * g * D : (4 * g + 4) * D],
                        )

        pending = None
        deferred_pool = []

        def flush_deferred_pool():
            for xsl, bc1, hslice, xin in deferred_pool:
                nc.gpsimd.tensor_tensor(
                    out=xsl, in0=xsl,
                    in1=bc1.to_broadcast([128, 2, D]), op=ALU.add,
                )
                nc.sync.dma_start(out=hslice, in_=xin)
            deferred_pool.clear()

        def chain(b):
            nonlocal pending
            q_sb, qsq = q_sbs[b], qsqs[b]
            # 1/||q|| = exp(-0.5 * ln(qsq0 + qsq1))
            inv_qn = sm.tile([1, 1], F32, tag="inv_qn")
            nc.vector.tensor_add(inv_qn, qsq[0:1, 0:1], qsq[0:1, 1:2])
            nc.scalar.activation(out=inv_qn, in_=inv_qn, func=AF.Ln)
            nc.scalar.activation(out=inv_qn, in_=inv_qn, func=AF.Exp, scale=-0.5)

            # qT[128, 8] via one-hot outer products (one PSUM group)
            ps_qt = ps4.tile([128, DCH], F32, tag="ps")
            for dc in range(DCH):
                nc.tensor.matmul(
                    ps_qt,
                    lhsT=q_sb[0:1, dc * 128 : (dc + 1) * 128],
                    rhs=e8[0:1, dc, :],
                    start=(dc == 0),
                    stop=(dc == DCH - 1),
                )
            qT_h = sm.tile([128, DCH], F16, tag="qTh")
            nc.scalar.copy(out=qT_h, in_=ps_qt)

            # scores row: sum_dc qT[:,dc].T @ protoT_n[dc]
            ps_s = [ps4.tile([1, 512], F32, tag="ps", name=f"ps_s{h}") for h in range(2)]
            for dc in range(DCH):
                for h in range(2):
                    nc.tensor.matmul(
                        ps_s[h],
                        lhsT=qT_h[:, dc : dc + 1],
                        rhs=protoT_sb[:, dc, h * 512 : (h + 1) * 512],
                        start=(dc == 0),
                        stop=(dc == DCH - 1),
                    )

            # e = exp(cos / ||q||) in fp16; top-k/threshold/softmax all
            # happen in e-space (exp is monotonic), halving the DVE cost
            e_row = sm.tile([1, P], F16, tag="erow", bufs=1)
            for h in range(2):
                nc.scalar.activation(
                    out=e_row[0:1, h * 512 : (h + 1) * 512],
                    in_=ps_s[h],
                    func=AF.Exp,
                    scale=inv_qn,
                )
            vals2 = sm.tile([1, 16], F16, tag="vals2")
            for h in range(2):
                nc.vector.max(
                    out=vals2[0:1, 8 * h : 8 * h + 8],
                    in_=e_row[0:1, h * 512 : (h + 1) * 512],
                )
            vals = sm.tile([1, 8], F16, tag="vals")
            nc.vector.max(out=vals, in_=vals2)
            den = sm.tile([1, 1], F32, tag="den")
            nc.vector.reduce_sum(
                out=den, in_=vals[0:1, 0:5], axis=mybir.AxisListType.X
            )
            coef = sm.tile([1, 1], F16, tag="coef")
            with nc.allow_low_precision(reason="0.1/den fits fp16 comfortably"):
                nc.vector.reciprocal(out=coef, in_=den)
                nc.scalar.mul(out=coef, in_=coef, mul=ALPHA)

            # wt row = (e >= e_t5) * e, fp16
            wt_h = sm.tile([1, P], F16, tag="wth", bufs=1)
            for h in range(2):
                nc.vector.scalar_tensor_tensor(
                    out=wt_h[0:1, h * 512 : (h + 1) * 512],
                    in0=e_row[0:1, h * 512 : (h + 1) * 512],
                    scalar=vals[0:1, 4:5],
                    in1=e_row[0:1, h * 512 : (h + 1) * 512],
                    op0=ALU.is_ge,
                    op1=ALU.mult,
                )

            # wtT[128, 8] via one-hot outer products, quantized to fp8
            ps_wt = ps4.tile([128, PCH], F32, tag="ps")
            for pc in range(PCH):
                nc.tensor.matmul(
                    ps_wt,
                    lhsT=wt_h[0:1, pc * 128 : (pc + 1) * 128],
                    rhs=e8[0:1, pc, :],
                    start=(pc == 0),
                    stop=(pc == PCH - 1),
                )
            wtT_h = sm.tile([128, PCH], F32, tag="wtTh")
            nc.scalar.copy(out=wtT_h, in_=ps_wt)

            # replicate each weight chunk across 128 columns (per-partition
            # scale copy) so the agg matmul directly produces the broadcast
            # [128, D] tile -- and with M=128 the fp8 DoubleRow mode is legal
            wt_rep = sm.tile([128, PCH, 128], F8, tag="wtrep", bufs=1)
            for pc in range(PCH):
                nc.scalar.activation(
                    out=wt_rep[:, pc, :], in_=ones128, func=AF.Copy,
                    scale=wtT_h[:, pc : pc + 1],
                )
            # coef broadcast to all partitions (1-cycle ones outer product)
            ps_c = ps4.tile([128, 1], F32, tag="ps")
            nc.tensor.matmul(ps_c, lhsT=ones_row, rhs=coef, start=True, stop=True)
            coef_rep = sm.tile([128, 1], F32, tag="coefr")
            nc.scalar.copy(out=coef_rep, in_=ps_c)

            # fused agg+broadcast: bc[q, d] = sum_p wt[p] * proto8[p, d],
            # identical across q; fp8 DoubleRow halves the PE cycles
            bc_ps = ps_bc.tile([128, D], F32, tag="bc")
            bc_h = sm.tile([128, D], F16, tag="bch")
            for h in range(2):
                for j in range(PCH // 2):
                    nc.tensor.matmul(
                        bc_ps[:, h * 512 : (h + 1) * 512],
                        lhsT=wt_rep[:, 2 * j : 2 * j + 2, :],
                        rhs=proto8[:, 2 * j : 2 * j + 2, h * 512 : (h + 1) * 512],
                        start=(j == 0),
                        stop=(j == PCH // 2 - 1),
                        perf_mode=mybir.MatmulPerfMode.DoubleRow,
                    )
                nc.scalar.activation(
                    out=bc_h[:, h * 512 : (h + 1) * 512],
                    in_=bc_ps[:, h * 512 : (h + 1) * 512],
                    func=AF.Copy,
                    scale=coef_rep,
                )

            if NO_PENDING:
                emit_adds(b, bc_h)
            else:
                # previous batch's adds/stores AFTER this chain: the DVE
                # sequencer reaches the next chain's ops before the add burst
                if pending is not None:
                    emit_adds(*pending)
                pending = (b, bc_h)

        # ---- emission: fp8 proto table first (lands ~6 us, so the whole
        # norm/transpose prep chain and chain(0) run ~5 us earlier); the
        # second proto prep under-runs the ring by ~0.7 us once ----
        if PROTOS_FIRST:
            # bulk first: its longer desc-gen starts at t=0 so the ring
            # under-runs only ~0.2 us after the head transfer drains
            nc.gpsimd.dma_start(out=proto8[:, 1:PCH, :], in_=protos[128:P, :])
            nc.gpsimd.dma_start(out=proto8[NHEAD:128, 0, :], in_=protos[NHEAD:128, :])
            load_tile(0, 0)
        else:
            load_tile(0, 0)
            nc.gpsimd.dma_start(out=proto8[NHEAD:128, 0, :], in_=protos[NHEAD:128, :])
            nc.gpsimd.dma_start(out=proto8[:, 1:PCH, :], in_=protos[128:P, :])
        make_identity(nc, ident)
        nc.vector.memset(ones128, 1.0)
        nc.vector.memset(e8, 0.0)
        for j in range(8):
            nc.vector.memset(e8[0:1, j, j : j + 1], 1.0)
        nc.vector.memset(ones_col, 1.0)
        nc.vector.memset(ones_row, 1.0)
        for c in range(PCH):
            emit_sumsq(c)
        emit_inv_pnorm()
        load_batch(0, skip_first=True)
        for c in range(PCH):
            emit_protoT(c)
        load_batch(1)
        fold_q(0)
        chain(0)
        load_batch(2)
        fold_q(1)
        chain(1)
        # chain(2) before loads(3): batch-3's query runs on PE, so nothing
        # in chain(2) waits on batch-3 state and the chain cascade stays
        # ahead of the store drain
        fold_q(2)
        chain(2)
        load_batch(3)
        flush_deferred_pool()
        fold_q(3)
        chain(3)
        if not NO_PENDING:
            emit_adds(*pending)


def build_nc(repeat=1):
    # big SWDGE desc ring: every load's desc-gen completes (and claims its
    # DMA FIFO slot) long before the first store is ready, so stores can
    # never jump ahead of the remaining loads and delay the last batch
    nc = bacc.Bacc("TRN2", target_bir_lowering=False)
    x = nc.dram_tensor("x", [BLOC, L, D], F32, kind="ExternalInput")
    protos = nc.dram_tensor("prototypes", [P, D], F32, kind="ExternalInput")
    # fp16 output buffer: the result is computed in fp16 anyway, so storing
    # fp16 halves HBM write traffic; the host upcasts after gathering
    out = nc.dram_tensor("out", [BLOC, L, D], F16, kind="ExternalOutput")
    with tile.TileContext(nc) as tc, ExitStack() as ctx:
        _kernel(tc, ctx, x[:], protos[:], out[:], repeat=repeat)
    nc.finalize()
    return nc


def kernel(x, prototypes):
    x = np.ascontiguousarray(x, dtype=np.float32)
    prototypes = np.ascontiguousarray(prototypes, dtype=np.float32)
    assert x.shape == (B, L, D) and prototypes.shape == (P, D)
    nc = build_nc()
    in_maps = [
        {"x": x[c * BLOC : (c + 1) * BLOC], "prototypes": prototypes}
        for c in range(NCORES)
    ]
    res = run_bass_kernel_spmd(nc, in_maps, core_ids=list(range(NCORES)))
    full = np.concatenate([r["out"] for r in res.results], axis=0)
    return full.astype(np.float32)


# revision 47
# speedup vs baseline: 1.0247x; 1.0247x over previous
"""Trainium2 Bass kernel for nn_BPBookLayer (retrieval_knn).

Computation (per full input):
  query = mean(x, axis=1)                         [B, D]
  scores = cos_sim(query, prototypes)             [B, P]
  top5 -> softmax -> agg = attn @ protos[top5]    [B, D]
  out = x + 0.1 * agg[:, None, :]

Sharding: data-parallel over batch B=32 across 8 cores (4 batches/core),
prototypes replicated.

All DMA serializes on one ~360 GB/s device charged by out-side bytes, so
the kernel is bound by x in fp16 (16.8 MB) + out fp16 (16.8 MB) + the
1 MB fp8 proto table: ~96.4 us of transfer. Everything else is scheduled
to hide under that stream; the layout below keeps the store producer
ahead of the drain with zero mid-stream starvation beyond ~2 us.

Per-core design:
 - x loads f32 HBM -> fp16 SBUF (gpsimd casting DMA) as 8 whole-half-
   batch tiles [128, 8*D] (TROWS=1024): only 10 SWDGE DMAs total, so
   every load's desc-gen completes (and claims its DMA-engine FIFO slot)
   within ~12 us -- stores can never jump ahead of pending loads and
   push the last batch's tiles (and its whole chain) later.
 - prototypes load fp8-e4m3 in two early bulk DMAs (bulk first: its
   desc-gen starts at t=0); a 32-row f32 HWDGE head covers the window
   before SWDGE dispatches. 32 = SBUF quadrant size: engine ops on the
   [32:128] remainder must start on a 32-partition boundary (and span
   one quadrant) or the BIR verifier rejects them.
 - out stores fp16 (host upcasts); each batch is stored in 2-subrow
   quarters right behind its residual adds so production stays granular.
 - protoT = diag(1/||p||)-scaled transposes of the fp8 chunks (fp8 lhsT
   x fp16 diag through the f32 PE path), 4 chunks per PSUM bank. Norms:
   Act Square-accumulate straight off the fp8 table, then
   1/||p|| = exp(-0.5*ln(sumsq)) -- with Copy/Square/Ln/Exp all in ONE
   act-func set (explicit LoadActFuncSet: the auto-insert pass is greedy
   per-function and would thrash tables between Ln and Exp).
 - query accumulation: batches 0-2 on DVE (fp16 tensor_tensor folds of
   each tile into a [128, 4*D] accumulator, gated per tile), batch 3 on
   PE (ones-column matmuls riding the loads) so the post-load DVE window
   holds only adds + chain work. chain(2) is emitted before loads(3) so
   the chain cascade never queues behind batch-3-gated ops.
 - per-batch chain: scores row = sum_dc qT[:,dc].T @ protoT_n[dc]
   (16 matmuls); e = exp(cos/||q||) in fp16 and top-5/threshold/softmax
   all happen in e-space (exp is monotonic, halving DVE top-k cost);
   wt quantizes to fp8 while replicating across 128 columns (Act
   per-partition-scale copy), and agg+broadcast fuse into 8 fp8
   DoubleRow matmuls producing the [128, D] broadcast tile directly
   (M=128 makes DoubleRow legal; the 0.1/softmax-denominator scale is
   applied via a PE-broadcast coefficient column on the PSUM copy-out).
 - residual adds in-place on DVE fp16; each batch's adds are emitted
   after the NEXT chain (pending trick) so the DVE sequencer reaches
   chain ops first; the tail quarters of late batches ride the gpsimd
   engine in parallel.
"""

from contextlib import ExitStack

import numpy as np

import concourse.bacc as bacc
import concourse.bass as bass
import concourse.tile as tile
from concourse import mybir
from concourse.bass_utils import run_bass_kernel_spmd
from concourse.masks import make_identity

F32 = mybir.dt.float32
F16 = mybir.dt.float16
F8 = mybir.dt.float8e4
AF = mybir.ActivationFunctionType
ALU = mybir.AluOpType

B, L, D, P = 32, 2048, 1024, 1024
NCORES = 8
BLOC = B // NCORES  # batches per core
TROWS = 1024  # L rows per x tile
TSUB = TROWS // 128
NT = L // TROWS     # x tiles per batch
DCH = D // 128      # d chunks
PCH = P // 128      # p chunks
XBUFS = BLOC * NT   # all x tiles resident
ALPHA = 0.1
NHEAD = 32          # proto rows riding the f32 HWDGE head transfer
# (32 = SBUF partition quadrant: engine ops on the [NHEAD:128] remainder
#  must start on a 32-partition boundary or the BIR verifier rejects them)
POOL_ADDS = True
PROTOS_FIRST = True
FINE_ADDS = True
NO_PENDING = False
DVE_SUMSQ_FROM = 5
PE_Q3 = True      # batch-3 query reduction on PE instead of DVE
POOL_B3_EXTRA = False  # gpsimd also takes batch-3's second-to-last tile


def _ln_exp_set_id(arch):
    """Index of the act-func set holding Ln+Exp+Square+Copy together."""
    try:
        from concourse.hw_specs import get_activation_tables

        need = {AF.Ln, AF.Exp, AF.Square, AF.Copy}
        for i, funcs in enumerate(get_activation_tables(arch).values()):
            if need <= funcs:
                return i
    except Exception:
        pass
    return 6  # natural_log_exp_and_others in the stock act_info.json


def _kernel(tc, ctx, x, protos, out, repeat=1):
    nc = tc.nc

    singles = ctx.enter_context(tc.tile_pool(name="singles", bufs=1))
    xp = ctx.enter_context(tc.tile_pool(name="xp", bufs=XBUFS))
    sm = ctx.enter_context(tc.tile_pool(name="sm", bufs=2))
    ps4 = ctx.enter_context(tc.tile_pool(name="ps4", bufs=2, space="PSUM"))
    psq = ctx.enter_context(tc.tile_pool(name="psq", bufs=2, space="PSUM"))
    ps_bc = ctx.enter_context(tc.tile_pool(name="ps_bc", bufs=1, space="PSUM"))

    for _rep in range(repeat):
        proto8 = singles.tile([128, PCH, D], F8)
        protoT_sb = singles.tile([128, DCH, P], F16)  # normalized transpose
        inv_pnorm = singles.tile([128, PCH], F32)
        pnorm_sq = singles.tile([128, PCH], F32)

        ident = singles.tile([128, 128], F16)
        ones128 = singles.tile([128, 128], F16)
        e8 = singles.tile([1, 8, 8], F16)
        ones_col = singles.tile([128, 1], F16)
        ones_row = singles.tile([1, 128], F16)

        # one act table for the whole kernel (Copy/Square/Ln/Exp)
        nc.scalar.add_instruction(
            mybir.InstLoadActFuncSet(
                name=nc.get_next_instruction_name(),
                ins=[],
                outs=[],
                act_func_set_id=_ln_exp_set_id(nc.m.arch),
            )
        )

        # ---- f32 proto head on HWDGE (covers SWDGE desc-gen startup),
        # then the whole fp8 proto table in two early SWDGE transfers ----
        proto0_f32 = singles.tile([NHEAD, D], F32)
        nc.sync.dma_start(out=proto0_f32, in_=protos[0:NHEAD, :])
        # agg reads the fp8 table incl. the head rows
        nc.scalar.copy(out=proto8[0:NHEAD, 0, :], in_=proto0_f32)

        sq_scratch = sm.tile([128, D], F16, tag="sqs", bufs=1)
        sq_scratch2 = sm.tile([128, D], F16, tag="sqs2", bufs=1)

        def emit_sumsq(c):
            """squared norms on Act straight from the fp8 table"""
            if c == 0:
                nc.scalar.activation(
                    out=sq_scratch[0:NHEAD, :], in_=proto0_f32, func=AF.Square,
                    accum_out=pnorm_sq[0:NHEAD, 0:1],
                )
                for q0 in range(NHEAD, 128, 32):  # quadrant-sized slices:
                    # engine APs with base partition != 0 may span <= 32
                    nc.scalar.activation(
                        out=sq_scratch[q0 : q0 + 32, :],
                        in_=proto8[q0 : q0 + 32, 0, :],
                        func=AF.Square, accum_out=pnorm_sq[q0 : q0 + 32, 0:1],
                    )
            elif c >= DVE_SUMSQ_FROM:
                # late chunks on DVE with standard ops (mult + reduce):
                # the custom fused reduce crashes the runtime on fp8 inputs
                nc.vector.tensor_tensor(
                    out=sq_scratch2, in0=proto8[:, c, :], in1=proto8[:, c, :],
                    op=ALU.mult,
                )
                nc.vector.reduce_sum(
                    out=pnorm_sq[:, c : c + 1], in_=sq_scratch2,
                    axis=mybir.AxisListType.X,
                )
            else:
                nc.scalar.activation(
                    out=sq_scratch, in_=proto8[:, c, :], func=AF.Square,
                    accum_out=pnorm_sq[:, c : c + 1],
                )

        def emit_inv_pnorm():
            """1/||p|| per partition: ln/exp on Act, all chunks at once."""
            nc.scalar.activation(out=pnorm_sq, in_=pnorm_sq, func=AF.Ln)
            nc.scalar.activation(
                out=inv_pnorm, in_=pnorm_sq, func=AF.Exp, scale=-0.5
            )

        xt = [[None] * NT for _ in range(BLOC)]
        acc = [None] * BLOC

        pe_q = {}

        def load_tile(b, i):
            t_ = xp.tile([128, TSUB * D], F16, tag="x", name=f"x{b}_{i}")
            xt[b][i] = t_
            nc.gpsimd.dma_start(
                out=t_,
                in_=x[b, TROWS * i : TROWS * (i + 1), :].rearrange(
                    "(p t) d -> p (t d)", p=128
                ),
            )
            if b == BLOC - 1 and PE_Q3:
                # last batch: query accumulates on PE (tile-gated matmuls)
                # so the post-load DVE window holds only adds + chain bits
                if i == 0:
                    pe_q[b] = [
                        psq.tile([1, 512], F32, tag="q", bufs=4,
                                 name=f"ps_q{b}_{h}")
                        for h in range(2)
                    ]
                for t in range(TSUB):
                    for h in range(2):
                        nc.tensor.matmul(
                            pe_q[b][h],
                            lhsT=ones_col,
                            rhs=xt[b][i][:, t * D + h * 512 : t * D + h * 512 + 512],
                            start=(i == 0 and t == 0),
                            stop=(i == NT - 1 and t == TSUB - 1),
                        )
                return
            # earlier batches: query accumulation rides DVE behind the
            # loads, folding each 8-subrow tile into a [128, 4*D] accumulator
            HALF = TSUB * D // 2
            if i == 0:
                a_ = sm.tile([128, HALF], F16, tag="qacc", bufs=2,
                             name=f"acc{b}")
                acc[b] = a_
                nc.vector.tensor_add(a_, t_[:, 0:HALF], t_[:, HALF:])
            else:
                nc.vector.tensor_add(acc[b], acc[b], t_[:, 0:HALF])
                nc.vector.tensor_add(acc[b], acc[b], t_[:, HALF:])

        def load_batch(b, skip_first=False):
            for i in range(1 if skip_first else 0, NT):
                load_tile(b, i)

        def emit_protoT(c):
            # diag(1/||p||)-scaled transposes of the fp8 chunk (fp8 lhsT x
            # fp16 diag is exact through the f32 PE path), 4 per PSUM bank
            diag_c = sm.tile([128, 128], F16, tag="diag", bufs=2, name=f"dg{c}")
            nc.scalar.activation(
                out=diag_c, in_=ident, func=AF.Copy,
                scale=inv_pnorm[:, c : c + 1],
            )
            for half in range(2):
                pst = ps4.tile([128, 4, 128], F32, tag="ps")
                for j in range(4):
                    dc = half * 4 + j
                    nc.tensor.matmul(
                        pst[:, j, :],
                        lhsT=proto8[:, c, dc * 128 : (dc + 1) * 128],
                        rhs=diag_c,
                        start=(j == 0),
                        stop=(j == 3),
                        skip_group_check=True,
                    )
                dst = protoT_sb[:, half * 4 : half * 4 + 4, c * 128 : (c + 1) * 128]
                if half == 0:
                    nc.scalar.copy(out=dst, in_=pst)
                else:
                    nc.vector.tensor_copy(dst, pst)

        q_sbs, qsqs = [], []

        def fold_q(b):
            if b in pe_q:
                ps_q = pe_q[b]  # accumulated during the loads
            else:
                ps_q = [
                    psq.tile([1, 512], F32, tag="q", bufs=4, name=f"ps_q{b}_{h}")
                    for h in range(2)
                ]
                for g in range(TSUB // 2):
                    for h in range(2):
                        nc.tensor.matmul(
                            ps_q[h],
                            lhsT=ones_col,
                            rhs=acc[b][:, g * D + h * 512 : g * D + h * 512 + 512],
                            start=(g == 0),
                            stop=(g == TSUB // 2 - 1),
                        )
            q_sb = sm.tile([1, D], F16, tag="q", bufs=2, name=f"q_sb{b}")
            for h in range(2):
                nc.scalar.copy(out=q_sb[0:1, h * 512 : (h + 1) * 512], in_=ps_q[h])
            qsq_sc = sm.tile([1, 512], F16, tag="qsq_sc", bufs=1)
            qsq = sm.tile([1, 2], F32, tag="qsq", bufs=2, name=f"qsq{b}")
            for h in range(2):
                nc.scalar.activation(
                    out=qsq_sc, in_=ps_q[h], func=AF.Square,
                    accum_out=qsq[0:1, h : h + 1],
                )
            q_sbs.append(q_sb)
            qsqs.append(qsq)

        def emit_adds(b, bc_h):
            bc1 = bc_h.rearrange("p (o d) -> p o d", o=1)
            for i in range(NT):
                xv = xt[b][i].rearrange("p (t d) -> p t d", d=D)
                hbm = out[b, TROWS * i : TROWS * (i + 1), :].rearrange(
                    "(p t) d -> p t d", p=128
                )
                if i == 0:
                    # first tile in 2-subrow quarters: stores launch early
                    for g in range(4):
                        sl = slice(2 * g, 2 * g + 2)
                        nc.vector.tensor_tensor(
                            out=xv[:, sl, :], in0=xv[:, sl, :],
                            in1=bc1.to_broadcast([128, 2, D]), op=ALU.add,
                        )
                        nc.sync.dma_start(
                            out=hbm[:, sl, :],
                            in_=xt[b][i][:, 2 * g * D : (2 * g + 2) * D],
                        )
                elif FINE_ADDS:
                    for g in range(4):
                        sl = slice(2 * g, 2 * g + 2)
                        # tail quarters ride gpsimd for late batches; Pool is
                        # in-order, so any pool op emitted before the last
                        # load desc-gen would stall it -- batch 1's pool
                        # quarters are deferred until after load_batch(3)
                        pool_this = POOL_ADDS and b >= 2 and g >= 2
                        if pool_this and b == 1:
                            deferred_pool.append(
                                (xv[:, sl, :], bc1,
                                 hbm[:, sl, :],
                                 xt[b][i][:, 2 * g * D : (2 * g + 2) * D])
                            )
                            continue
                        eng = nc.gpsimd if pool_this else nc.vector
                        eng.tensor_tensor(
                            out=xv[:, sl, :], in0=xv[:, sl, :],
                            in1=bc1.to_broadcast([128, 2, D]), op=ALU.add,
                        )
                        nc.sync.dma_start(
                            out=hbm[:, sl, :],
                            in_=xt[b][i][:, 2 * g * D : (2 * g + 2) * D],
                        )
                else:
                    for g in range(2):
                        sl = slice(4 * g, 4 * g + 4)
                        # second half rides gpsimd for late batches (their
                        # adds are emitted after every load desc-gen)
                        pool_this = POOL_ADDS and b >= 2 and g == 1
                        eng = nc.gpsimd if pool_this else nc.vector
                        eng.tensor_tensor(
                            out=xv[:, sl, :], in0=xv[:, sl, :],
                            in1=bc1.to_broadcast([128, 4, D]), op=ALU.add,
                        )
                        nc.sync.dma_start(
                            out=hbm[:, sl, :],
                            in_=xt[b][i][:, 4 * g * D : (4 * g + 4) * D],
                        )

        pending = None
        deferred_pool = []

        def flush_deferred_pool():
            for xsl, bc1, hslice, xin in deferred_pool:
                nc.gpsimd.tensor_tensor(
                    out=xsl, in0=xsl,
                    in1=bc1.to_broadcast([128, 2, D]), op=ALU.add,
                )
                nc.sync.dma_start(out=hslice, in_=xin)
            deferred_pool.clear()

        def chain(b):
            nonlocal pending
            q_sb, qsq = q_sbs[b], qsqs[b]
            # 1/||q|| = exp(-0.5 * ln(qsq0 + qsq1))
            inv_qn = sm.tile([1, 1], F32, tag="inv_qn")
            nc.vector.tensor_add(inv_qn, qsq[0:1, 0:1], qsq[0:1, 1:2])
            nc.scalar.activation(out=inv_qn, in_=inv_qn, func=AF.Ln)
            nc.scalar.activation(out=inv_qn, in_=inv_qn, func=AF.Exp, scale=-0.5)

            # qT[128, 8] via one-hot outer products (one PSUM group)
            ps_qt = ps4.tile([128, DCH], F32, tag="ps")
            for dc in range(DCH):
                nc.tensor.matmul(
                    ps_qt,
                    lhsT=q_sb[0:1, dc * 128 : (dc + 1) * 128],
                    rhs=e8[0:1, dc, :],
                    start=(dc == 0),
                    stop=(dc == DCH - 1),
                )
            qT_h = sm.tile([128, DCH], F16, tag="qTh")
            nc.scalar.copy(out=qT_h, in_=ps_qt)

            # scores row: sum_dc qT[:,dc].T @ protoT_n[dc]
            ps_s = [ps4.tile([1, 512], F32, tag="ps", name=f"ps_s{h}") for h in range(2)]
            for dc in range(DCH):
                for h in range(2):
                    nc.tensor.matmul(
                        ps_s[h],
                        lhsT=qT_h[:, dc : dc + 1],
                        rhs=protoT_sb[:, dc, h * 512 : (h + 1) * 512],
                        start=(dc == 0),
                        stop=(dc == DCH - 1),
                    )

            # e = exp(cos / ||q||) in fp16; top-k/threshold/softmax all
            # happen in e-space (exp is monotonic), halving the DVE cost
            e_row = sm.tile([1, P], F16, tag="erow", bufs=1)
            for h in range(2):
                nc.scalar.activation(
                    out=e_row[0:1, h * 512 : (h + 1) * 512],
                    in_=ps_s[h],
                    func=AF.Exp,
                    scale=inv_qn,
                )
            vals2 = sm.tile([1, 16], F16, tag="vals2")
            for h in range(2):
                nc.vector.max(
                    out=vals2[0:1, 8 * h : 8 * h + 8],
                    in_=e_row[0:1, h * 512 : (h + 1) * 512],
                )
            vals = sm.tile([1, 8], F16, tag="vals")
            nc.vector.max(out=vals, in_=vals2)
            den = sm.tile([1, 1], F32, tag="den")
            nc.vector.reduce_sum(
                out=den, in_=vals[0:1, 0:5], axis=mybir.AxisListType.X
            )
            coef = sm.tile([1, 1], F16, tag="coef")
            with nc.allow_low_precision(reason="0.1/den fits fp16 comfortably"):
                nc.vector.reciprocal(out=coef, in_=den)
                nc.scalar.mul(out=coef, in_=coef, mul=ALPHA)

            # wt row = (e >= e_t5) * e, fp16
            wt_h = sm.tile([1, P], F16, tag="wth", bufs=1)
            for h in range(2):
                nc.vector.scalar_tensor_tensor(
                    out=wt_h[0:1, h * 512 : (h + 1) * 512],
                    in0=e_row[0:1, h * 512 : (h + 1) * 512],
                    scalar=vals[0:1, 4:5],
                    in1=e_row[0:1, h * 512 : (h + 1) * 512],
                    op0=ALU.is_ge,
                    op1=ALU.mult,
                )

            # wtT[128, 8] via one-hot outer products, quantized to fp8
            ps_wt = ps4.tile([128, PCH], F32, tag="ps")
            for pc in range(PCH):
                nc.tensor.matmul(
                    ps_wt,
                    lhsT=wt_h[0:1, pc * 128 : (pc + 1) * 128],
                    rhs=e8[0:1, pc, :],
                    start=(pc == 0),
                    stop=(pc == PCH - 1),
                )
            wtT_h = sm.tile([128, PCH], F32, tag="wtTh")
            nc.scalar.copy(out=wtT_h, in_=ps_wt)

            # replicate each weight chunk across 128 columns (per-partition
            # scale copy) so the agg matmul directly produces the broadcast
            # [128, D] tile -- and with M=128 the fp8 DoubleRow mode is legal
            wt_rep = sm.tile([128, PCH, 128], F8, tag="wtrep", bufs=1)
            for pc in range(PCH):
                nc.scalar.activation(
                    out=wt_rep[:, pc, :], in_=ones128, func=AF.Copy,
                    scale=wtT_h[:, pc : pc + 1],
                )
            # coef broadcast to all partitions (1-cycle ones outer product)
            ps_c = ps4.tile([128, 1], F32, tag="ps")
            nc.tensor.matmul(ps_c, lhsT=ones_row, rhs=coef, start=True, stop=True)
            coef_rep = sm.tile([128, 1], F32, tag="coefr")
            nc.scalar.copy(out=coef_rep, in_=ps_c)

            # fused agg+broadcast: bc[q, d] = sum_p wt[p] * proto8[p, d],
            # identical across q; fp8 DoubleRow halves the PE cycles
            bc_ps = ps_bc.tile([128, D], F32, tag="bc")
            bc_h = sm.tile([128, D], F16, tag="bch")
            for h in range(2):
                for j in range(PCH // 2):
                    nc.tensor.matmul(
                        bc_ps[:, h * 512 : (h + 1) * 512],
                        lhsT=wt_rep[:, 2 * j : 2 * j + 2, :],
                        rhs=proto8[:, 2 * j : 2 * j + 2, h * 512 : (h + 1) * 512],
                        start=(j == 0),
                        stop=(j == PCH // 2 - 1),
                        perf_mode=mybir.MatmulPerfMode.DoubleRow,
                    )
                nc.scalar.activation(
                    out=bc_h[:, h * 512 : (h + 1) * 512],
                    in_=bc_ps[:, h * 512 : (h + 1) * 512],
                    func=AF.Copy,
                    scale=coef_rep,
                )

            if NO_PENDING:
                emit_adds(b, bc_h)
            else:
                # previous batch's adds/stores AFTER this chain: the DVE
                # sequencer reaches the next chain's ops before the add burst
                if pending is not None:
                    emit_adds(*pending)
                pending = (b, bc_h)

        # ---- emission: fp8 proto table first (lands ~6 us, so the whole
        # norm/transpose prep chain and chain(0) run ~5 us earlier); the
        # second proto prep under-runs the ring by ~0.7 us once ----
        if PROTOS_FIRST:
            # bulk first: its longer desc-gen starts at t=0 so the ring
            # under-runs only ~0.2 us after the head transfer drains
            nc.gpsimd.dma_start(out=proto8[:, 1:PCH, :], in_=protos[128:P, :])
            nc.gpsimd.dma_start(out=proto8[NHEAD:128, 0, :], in_=protos[NHEAD:128, :])
            load_tile(0, 0)
        else:
            load_tile(0, 0)
            nc.gpsimd.dma_start(out=proto8[NHEAD:128, 0, :], in_=protos[NHEAD:128, :])
            nc.gpsimd.dma_start(out=proto8[:, 1:PCH, :], in_=protos[128:P, :])
        make_identity(nc, ident)
        nc.vector.memset(ones128, 1.0)
        nc.vector.memset(e8, 0.0)
        for j in range(8):
            nc.vector.memset(e8[0:1, j, j : j + 1], 1.0)
        nc.vector.memset(ones_col, 1.0)
        nc.vector.memset(ones_row, 1.0)
        for c in range(PCH):
            emit_sumsq(c)
        emit_inv_pnorm()
        load_batch(0, skip_first=True)
        for c in range(PCH):
            emit_protoT(c)
        load_batch(1)
        fold_q(0)
        chain(0)
        load_batch(2)
        fold_q(1)
        chain(1)
        # chain(2) before loads(3): batch-3's query runs on PE, so nothing
        # in chain(2) waits on batch-3 state and the chain cascade stays
        # ahead of the store drain
        fold_q(2)
        chain(2)
        load_batch(3)
        flush_deferred_pool()
        fold_q(3)
        chain(3)
        if not NO_PENDING:
            emit_adds(*pending)


def build_nc(repeat=1):
    # big SWDGE desc ring: every load's desc-gen completes (and claims its
    # DMA FIFO slot) long before the first store is ready, so stores can
    # never jump ahead of the remaining loads and delay the last batch
    nc = bacc.Bacc("TRN2", target_bir_lowering=False)
    x = nc.dram_tensor("x", [BLOC, L, D], F32, kind="ExternalInput")
    protos = nc.dram_tensor("prototypes", [P, D], F32, kind="ExternalInput")
    # fp16 output buffer: the result is computed in fp16 anyway, so storing
    # fp16 halves HBM write traffic; the host upcasts after gathering
    out = nc.dram_tensor("out", [BLOC, L, D], F16, kind="ExternalOutput")
    with tile.TileContext(nc) as tc, ExitStack() as ctx:
        _kernel(tc, ctx, x[:], protos[:], out[:], repeat=repeat)
    nc.finalize()
    return nc


def kernel(x, prototypes):
    x = np.ascontiguousarray(x, dtype=np.float32)
    prototypes = np.ascontiguousarray(prototypes, dtype=np.float32)
    assert x.shape == (B, L, D) and prototypes.shape == (P, D)
    nc = build_nc()
    in_maps = [
        {"x": x[c * BLOC : (c + 1) * BLOC], "prototypes": prototypes}
        for c in range(NCORES)
    ]
    res = run_bass_kernel_spmd(nc, in_maps, core_ids=list(range(NCORES)))
    full = np.concatenate([r["out"] for r in res.results], axis=0)
    return full.astype(np.float32)
